# revision 29
# baseline (speedup 1.0000x reference)
"""PointNet feature interpolation (3-NN inverse-distance) Trainium2 kernel.

Problem (per batch b of 8, one NeuronCore each):
  xyz1:    [3, N=8192]   source point coords
  xyz2:    [3, S=2048]   query point coords
  points1: [D=256, N]    source features
  out:     [D, S]        interpolated features

Device algorithm per core (v5):
  1. M[s, n] = 512*(2*x2_s.x1_n - |x1_n|^2 - |x2_s|^2)  (= 512 * -dist^2)
     as one K=28 fp8e4 DoubleRow matmul (2x PE rate); coords pre-scaled
     by 32 and split into 3 fp8 terms with all i+j<=4 cross products;
     norms as base-224 fp8 digit rows.  (Fallback flag "bf16mm": K=30
     bf16 split matmul, exactly the proven v3 scheme.)
  2. PSUM -> SBUF copy casts to bf16; pairwise max over contiguous
     16-point blocks folds 8192 -> 512 block maxima; vector.max /
     max_index give the top-8 blocks per query row (HW dedupes ties).
  3. Per half (8 chunks): block ids are repacked on-chip into the
     dma_gather wrapped-index layout (idxs[q, j*8+a] = bi[16a+q, j]);
     one dma_gather per chunk (J=8, 1024 desc) pulls the blocks'
     [2x,2y,2z,-n1] rows; candidates are recomputed exactly in fp32
     (cand = 2e - n1; per-row-constant n2 affects no ordering);
     vector.max gives top-3 values; equality-match against a gmap
     decodes global indices.
  4. d3 = n2 - cand for the top 3; inverse-distance weights; one
     dma_gather per chunk pair (J=8: 6 real + 2 pad) pulls bf16
     features; weighted sum in bf16; output stored [S, D] bf16 and
     transposed to [D, S] fp32 on host.
"""

import numpy as np
import ml_dtypes

B, N, S, D = 8, 8192, 2048, 256
P = 128
NCHUNK = S // P      # 16 query-row chunks per core
NT = 512             # matmul moving free dim (one PSUM bank)
KP = 14              # fp8 DoubleRow contraction partitions (28 rows)
K30 = 30             # bf16 fallback contraction rows
NBLK = 512           # blocks per row (16 consecutive points each)
BPTS = N // NBLK     # 16
NB = 8               # candidate blocks kept per row
NCAND = NB * BPTS    # 128 candidate points per row
H = 8                # chunks per half
NHALF = NCHUNK // H  # 2

_COMPILED = {}


def _build_bass(abl=()):
    import concourse.bass as bass
    import concourse.mybir as mybir
    import concourse.tile as tile
    from concourse import bacc

    f32 = mybir.dt.float32
    bf16 = mybir.dt.bfloat16
    fp8 = mybir.dt.float8e4
    u32 = mybir.dt.uint32
    u16 = mybir.dt.uint16
    i16 = mybir.dt.int16
    Alu = mybir.AluOpType
    X = mybir.AxisListType.X
    DR = mybir.MatmulPerfMode.DoubleRow
    bf16mm = "bf16mm" in abl

    nc = bacc.Bacc(None)
    if bf16mm:
        x2m = nc.dram_tensor("x2m", [K30, S], bf16, kind="ExternalInput")
        x1m = nc.dram_tensor("x1m", [K30, N], bf16, kind="ExternalInput")
    else:
        x2m = nc.dram_tensor("x2m", [KP, 2, S], fp8, kind="ExternalInput")
        x1m = nc.dram_tensor("x1m", [KP, 2, N], fp8, kind="ExternalInput")
    xblk = nc.dram_tensor("xblk", [NBLK, BPTS * 4], f32, kind="ExternalInput")
    x2q = nc.dram_tensor("x2q", [P, NCHUNK, 4], f32, kind="ExternalInput")
    n2t = nc.dram_tensor("n2t", [P, NCHUNK], f32, kind="ExternalInput")
    p1t = nc.dram_tensor("p1t", [N, D], bf16, kind="ExternalInput")
    outS = nc.dram_tensor("outS", [S, D], bf16, kind="ExternalOutput")
    if "dbg" in abl:
        bi_d = nc.dram_tensor("bi_d", [P, NCHUNK, NB], u32,
                              kind="ExternalOutput")
        nd_d = nc.dram_tensor("nd_d", [P, NCHUNK, NB, BPTS], f32,
                              kind="ExternalOutput")
        cv_d = nc.dram_tensor("cv_d", [P, NCHUNK, NB], f32,
                              kind="ExternalOutput")
        n3_d = nc.dram_tensor("n3_d", [P, NCHUNK, 3], u32,
                              kind="ExternalOutput")
        bm_d = nc.dram_tensor("bm_d", [P, NCHUNK, NBLK], bf16,
                              kind="ExternalOutput")
        fidx_d = nc.dram_tensor("fidx_d", [P, NCHUNK // 2, 8, 8], i16,
                                kind="ExternalOutput")
        gf_d = nc.dram_tensor("gf_d", [P, NCHUNK // 2, 8, D], bf16,
                              kind="ExternalOutput")
        w3_d = nc.dram_tensor("w3_d", [P, NCHUNK, 3], bf16,
                              kind="ExternalOutput")
        acc_d = nc.dram_tensor("acc_d", [P, NCHUNK, D], bf16,
                               kind="ExternalOutput")

    with tile.TileContext(nc) as tc:
        with (
            tc.tile_pool(name="const", bufs=1) as cpool,
            tc.tile_pool(name="negb", bufs=2) as nbpool,
            tc.tile_pool(name="tree", bufs=2) as tpool,
            tc.tile_pool(name="mm", bufs=2, space="PSUM") as mmpool,
            tc.tile_pool(name="half", bufs=1) as hpool,
            tc.tile_pool(name="small", bufs=2) as spool,
        ):
            if bf16mm:
                x2s = cpool.tile([K30, S], bf16)
                x1s = cpool.tile([K30, N], bf16)
            else:
                x2s = cpool.tile([KP, 2, S], fp8)
                x1s = cpool.tile([KP, 2, N], fp8)
            nc.sync.dma_start(x2s[:], x2m[:])
            nc.sync.dma_start(x1s[:], x1m[:])
            x2q_sb = cpool.tile([P, NCHUNK, 4], f32)
            nc.sync.dma_start(x2q_sb[:], x2q[:])
            n2_sb = cpool.tile([P, NCHUNK], f32)
            nc.sync.dma_start(n2_sb[:], n2t[:])
            # iota16[p, m] = m + 1 (bias so failed match decodes to -1)
            iota16 = cpool.tile([P, BPTS], u32)
            nc.gpsimd.iota(iota16[:], pattern=[[1, BPTS]], base=1,
                           channel_multiplier=0)

            for hf in range(NHALF):
                bi_all = hpool.tile([P, H, NB], u32, tag="bi")
                cv8 = hpool.tile([P, H, NB], f32, tag="cv8")

                # ---- pass 1: distances, block maxima, top-8 blocks ----
                for pair in range(H // 2):
                    negb = nbpool.tile([P, 2, N], bf16, tag="negb")
                    for half in range(2):
                        ci = hf * H + pair * 2 + half
                        for q in range(4):
                            ps = mmpool.tile([P, 2048], f32, tag="mm")
                            for j in range(4):
                                nt = q * 4 + j
                                if bf16mm:
                                    nc.tensor.matmul(
                                        ps[:, j * NT:(j + 1) * NT],
                                        lhsT=x2s[:, ci * P:(ci + 1) * P],
                                        rhs=x1s[:, nt * NT:(nt + 1) * NT],
                                        start=True, stop=True,
                                    )
                                else:
                                    nc.tensor.matmul(
                                        ps[:, j * NT:(j + 1) * NT],
                                        lhsT=x2s[:, :, ci * P:(ci + 1) * P],
                                        rhs=x1s[:, :, nt * NT:(nt + 1) * NT],
                                        start=True, stop=True,
                                        perf_mode=DR,
                                    )
                            nc.scalar.copy(
                                negb[:, half, q * 2048:(q + 1) * 2048], ps[:])

                    # pairwise max over contiguous 16-point blocks (both
                    # chunks of the pair in one op, bf16 2x mode)
                    nv = negb[:].rearrange("p h (j m) -> p h j m", m=BPTS)
                    ts = tpool.tile([P, 2, NBLK, 8], bf16, tag="ts")
                    nc.vector.tensor_tensor(
                        out=ts[:], in0=nv[:, :, :, 0:8], in1=nv[:, :, :, 8:16],
                        op=Alu.max)
                    nc.vector.tensor_tensor(
                        out=ts[:, :, :, 0:4], in0=ts[:, :, :, 0:4],
                        in1=ts[:, :, :, 4:8], op=Alu.max)
                    nc.vector.tensor_tensor(
                        out=ts[:, :, :, 0:2], in0=ts[:, :, :, 0:2],
                        in1=ts[:, :, :, 2:4], op=Alu.max)
                    bm = tpool.tile([P, 2, NBLK], bf16, tag="bm")
                    nc.vector.tensor_tensor(
                        out=bm[:], in0=ts[:, :, :, 0], in1=ts[:, :, :, 1],
                        op=Alu.max)

                    for half in range(2):
                        cc = pair * 2 + half
                        bv8 = spool.tile([P, 8], bf16, tag="bv8")
                        nc.vector.max(out=bv8[:], in_=bm[:, half])
                        nc.vector.max_index(out=bi_all[:, cc, :],
                                            in_max=bv8[:],
                                            in_values=bm[:, half])
                    if "dbg" in abl:
                        nc.sync.dma_start(
                            bm_d[:, hf * H + pair * 2:hf * H + pair * 2 + 2],
                            bm[:])

                # ---- block gathers: wrapped-idx prep + dma_gather/chunk ----
                bi16 = hpool.tile([P, H * NB], i16, tag="bi16")
                nc.vector.tensor_copy(bi16[:],
                                      bi_all[:].rearrange("p c k -> p (c k)"))
                bidx = hpool.tile([P, H, NB, 8], i16, tag="bidx")
                bw = bidx[:].rearrange("p c k a -> p (c k) a")
                for a in range(8):
                    nc.sync.dma_start(
                        bw[0:16, :, a],
                        bi16[16 * a:16 * (a + 1), :])
                for r in range(1, 8):
                    nc.sync.dma_start(
                        bidx[16 * r:16 * (r + 1)]
                        .rearrange("p c k a -> p (c k a)"),
                        bidx[0:16].rearrange("p c k a -> p (c k a)"))
                gxb = hpool.tile([P, H, NB, BPTS * 4], f32, tag="gxb")
                for cc in range(H):
                    nc.gpsimd.dma_gather(
                        out_ap=gxb[:, cc],
                        in_ap=xblk[:],
                        idxs_ap=bidx[:, cc].rearrange("p k a -> p (k a)"),
                        num_idxs=P * NB,
                        num_idxs_reg=P * NB,
                        elem_size=BPTS * 4,
                    )

                # ---- candidate distances: cand = 2e - n1 (fp32 exact) ----
                # gxb planes per block row: [2x(16) 2y(16) 2z(16) -n1(16)]
                negdc = hpool.tile([P, H, NB, BPTS], f32, tag="negdc")
                for cc in range(H):
                    gv = gxb[:, cc].rearrange("p k (q m) -> p k q m", q=4)
                    qv = (x2q_sb[:, hf * H + cc, :]
                          .unsqueeze(1).unsqueeze(3)
                          .to_broadcast([P, NB, 4, BPTS]))
                    nc.vector.tensor_tensor(out=gv, in0=gv, in1=qv,
                                            op=Alu.mult)
                    pv = gxb[:, cc].rearrange("p k (q m) -> p k m q", q=4)
                    nc.vector.tensor_reduce(out=negdc[:, cc], in_=pv, axis=X,
                                            op=Alu.add)
                    nc.vector.max(out=cv8[:, cc, :], in_=negdc[:, cc])

                # ---- index decode: match top-3 values, take gmap ----
                gm16 = hpool.tile([P, H, NB], u32, tag="gm16")
                nc.gpsimd.tensor_scalar(out=gm16[:], in0=bi_all[:],
                                        scalar1=BPTS, scalar2=None,
                                        op0=Alu.mult)
                gmap = hpool.tile([P, H, NB, BPTS], u32, tag="gmap")
                nc.gpsimd.tensor_tensor(
                    out=gmap[:],
                    in0=gm16[:].unsqueeze(3).to_broadcast([P, H, NB, BPTS]),
                    in1=iota16[:].unsqueeze(1).unsqueeze(2)
                        .to_broadcast([P, H, NB, BPTS]),
                    op=Alu.add)
                shq = [P, H, 3, NCAND]
                ndflat = negdc[:].rearrange("p c k m -> p c (k m)")
                gmflat = gmap[:].rearrange("p c k m -> p c (k m)")
                eqt = hpool.tile(shq, u32, tag="eqt")
                nc.vector.tensor_tensor(
                    out=eqt[:],
                    in0=ndflat.unsqueeze(2).to_broadcast(shq),
                    in1=cv8[:, :, 0:3].unsqueeze(3).to_broadcast(shq),
                    op=Alu.is_equal)
                nc.vector.tensor_tensor(
                    out=eqt[:], in0=eqt[:],
                    in1=gmflat.unsqueeze(2).to_broadcast(shq),
                    op=Alu.mult)
                n3 = hpool.tile([P, H, 3], u32, tag="n3")
                nc.vector.tensor_reduce(out=n3[:], in_=eqt[:], axis=X,
                                        op=Alu.max)
                nc.vector.tensor_scalar(out=n3[:], in0=n3[:], scalar1=1,
                                        scalar2=None, op0=Alu.subtract)

                # ---- weights: d3 = n2 - cand3 (+1e-8), inverse-distance ----
                d3 = hpool.tile([P, H, 3], f32, tag="d3")
                nc.vector.tensor_tensor(
                    out=d3[:],
                    in0=n2_sb[:, hf * H:(hf + 1) * H].unsqueeze(2)
                        .to_broadcast([P, H, 3]),
                    in1=cv8[:, :, 0:3],
                    op=Alu.subtract)
                nc.vector.tensor_scalar(out=d3[:], in0=d3[:], scalar1=1e-8,
                                        scalar2=None, op0=Alu.add)
                nc.vector.reciprocal(d3[:], d3[:])
                rsum = hpool.tile([P, H], f32, tag="rsum")
                nc.vector.tensor_reduce(out=rsum[:], in_=d3[:], axis=X,
                                        op=Alu.add)
                nc.vector.reciprocal(rsum[:], rsum[:])
                w3 = hpool.tile([P, H, 3], bf16, tag="w3")
                nc.vector.tensor_tensor(
                    out=w3[:], in0=d3[:],
                    in1=rsum[:].unsqueeze(2).to_broadcast([P, H, 3]),
                    op=Alu.mult)

                # ---- feature gathers (J=8 per chunk pair: 6 real + 2 pad) --
                n3p = hpool.tile([P, H // 2, 8], i16, tag="n3p")
                nc.vector.memset(n3p[:], -1)
                nc.vector.tensor_copy(
                    n3p[:, :, 0:6],
                    n3[:].rearrange("p (r h) k -> p r (h k)", h=2))
                fidx = hpool.tile([P, H // 2, 8, 8], i16, tag="fidx")
                fw = fidx[:].rearrange("p r j a -> p (r j) a")
                for a in range(8):
                    nc.sync.dma_start(
                        fw[0:16, :, a],
                        n3p[16 * a:16 * (a + 1)]
                        .rearrange("p r j -> p (r j)"))
                for r in range(1, 8):
                    nc.sync.dma_start(
                        fidx[16 * r:16 * (r + 1)]
                        .rearrange("p r j a -> p (r j a)"),
                        fidx[0:16].rearrange("p r j a -> p (r j a)"))
                gf = hpool.tile([P, H // 2, 8, D], bf16, tag="gf")
                nc.vector.memset(gf[:, :, 6:8, :], 0.0)
                for pr in range(H // 2):
                    nc.gpsimd.dma_gather(
                        out_ap=gf[:, pr],
                        in_ap=p1t[:],
                        idxs_ap=fidx[:, pr].rearrange("p j a -> p (j a)"),
                        num_idxs=P * 8,
                        num_idxs_reg=P * 6,
                        elem_size=D,
                    )

                # ---- interpolate: acc[p,c,d] = sum_k w3[k] * gf[k] ----
                acc = hpool.tile([P, H, D], bf16, tag="acc")
                av = acc[:].rearrange("p (r h) d -> p r h d", h=2)
                wv = w3[:].rearrange("p (r h) k -> p r h k", h=2)
                gg = gf[:, :, 0:6, :].rearrange("p r (h k) d -> p r h k d",
                                                h=2)
                sh3 = [P, H // 2, 2, D]
                nc.vector.tensor_tensor(
                    out=av, in0=gg[:, :, :, 0, :],
                    in1=wv[:, :, :, 0:1].to_broadcast(sh3), op=Alu.mult)
                gm = hpool.tile([P, H // 2, 2, D], bf16, tag="gm")
                for k in (1, 2):
                    nc.vector.tensor_tensor(
                        out=gm[:], in0=gg[:, :, :, k, :],
                        in1=wv[:, :, :, k:k + 1].to_broadcast(sh3),
                        op=Alu.mult)
                    nc.vector.tensor_tensor(out=av, in0=av, in1=gm[:],
                                            op=Alu.add)

                # rows s = (hf*H + cc)*P + p
                dst = (outS[:]
                       .rearrange("(c p) d -> p c d", p=P)
                       [:, hf * H:(hf + 1) * H, :])
                nc.sync.dma_start(dst, acc[:])

                if "dbg" in abl:
                    sl = slice(hf * H, (hf + 1) * H)
                    sl2 = slice(hf * (H // 2), (hf + 1) * (H // 2))
                    nc.sync.dma_start(
                        fidx_d[:, sl2].rearrange("p r j a -> p (r j a)"),
                        fidx[:].rearrange("p r j a -> p (r j a)"))
                    nc.sync.dma_start(
                        gf_d[:, sl2].rearrange("p r j d -> p (r j d)"),
                        gf[:].rearrange("p r j d -> p (r j d)"))
                    nc.sync.dma_start(bi_d[:, sl], bi_all[:])
                    nc.sync.dma_start(w3_d[:, sl], w3[:])
                    nc.sync.dma_start(
                        acc_d[:, sl].rearrange("p c d -> p (c d)"),
                        acc[:].rearrange("p c d -> p (c d)"))
                    nc.sync.dma_start(
                        nd_d[:, sl].rearrange("p c k m -> p c (k m)"),
                        negdc[:].rearrange("p c k m -> p c (k m)"))
                    nc.sync.dma_start(cv_d[:, sl], cv8[:])
                    nc.sync.dma_start(n3_d[:, sl], n3[:])

    nc.finalize()
    return nc


def _fp8_split3(x):
    """Split array into 3 float8_e4m3 terms summing to ~x."""
    f8 = ml_dtypes.float8_e4m3
    a = x.astype(f8)
    r = x - a.astype(np.float64)
    b = r.astype(f8)
    r2 = r - b.astype(np.float64)
    c = r2.astype(f8)
    return a, b, c


def _digit_rows(v):
    """v >= 0 (float64) -> [(digit fp8 array, scale)] with
    sum(digit*scale) ~ v (successive base-224 residual split)."""
    f8 = ml_dtypes.float8_e4m3
    rows = []
    r = v.astype(np.float64)
    for s in (224.0, 224.0, 224.0, 1.0, 1.0):
        d = (r / s).astype(f8)
        rows.append((d, s))
        r = r - d.astype(np.float64) * s
    return rows


def _host_matrices_fp8(xyz2b, xyz1b):
    """K=28 fp8 contraction rows: M = 512*(2e - n1 - n2)."""
    f8 = ml_dtypes.float8_e4m3
    x2 = xyz2b.astype(np.float64)   # [3, S]
    x1 = xyz1b.astype(np.float64)   # [3, N]
    n2 = (x2 * x2).sum(axis=0)
    n1 = (x1 * x1).sum(axis=0)
    Ls, Rs = [], []
    for c in range(3):
        a = _fp8_split3(32.0 * x2[c])
        b = _fp8_split3(32.0 * x1[c])
        for (i, j) in ((0, 0), (0, 1), (1, 0), (0, 2), (2, 0), (1, 1)):
            Ls.append(np.asarray(a[i], dtype=f8))
            Rs.append(np.asarray(b[j], dtype=f8))
    for d, s in _digit_rows(512.0 * n1):
        Ls.append(np.full(x2.shape[1], -s, dtype=f8))
        Rs.append(d)
    for d, s in _digit_rows(512.0 * n2):
        Ls.append(d)
        Rs.append(np.full(x1.shape[1], -s, dtype=f8))
    X2 = np.stack(Ls).reshape(KP, 2, -1)   # [14, 2, S]
    X1 = np.stack(Rs).reshape(KP, 2, -1)   # [14, 2, N]
    return X2, X1, n2.astype(np.float32), n1.astype(np.float32)


def _bf_split3(x):
    bf = ml_dtypes.bfloat16
    h = x.astype(bf)
    r = x - h.astype(np.float64)
    l = r.astype(bf)
    r2 = r - l.astype(np.float64)
    q = r2.astype(bf)
    return h, l, q


def _host_matrices_bf16(xyz2b, xyz1b):
    """K=30 bf16 contraction rows (v3 scheme): M = 2e - n1 - n2."""
    bf = ml_dtypes.bfloat16
    x2 = xyz2b.astype(np.float64)
    x1 = xyz1b.astype(np.float64)
    n2 = (x2 * x2).sum(axis=0)
    n1 = (x1 * x1).sum(axis=0)
    Srows, Nrows = [], []
    for c in range(3):
        h2, l2, q2 = _bf_split3(x2[c])
        h1, l1, q1 = _bf_split3(x1[c])
        th2 = (2.0 * h2.astype(np.float64)).astype(bf)
        tl2 = (2.0 * l2.astype(np.float64)).astype(bf)
        tq2 = (2.0 * q2.astype(np.float64)).astype(bf)
        for a, b_ in ((th2, h1), (th2, l1), (tl2, h1), (th2, q1),
                      (tq2, h1), (tl2, l1), (tl2, q1), (tq2, l1)):
            Srows.append(a)
            Nrows.append(b_)
    ones_s = np.ones(x2.shape[1], dtype=bf)
    ones_n = np.ones(x1.shape[1], dtype=bf)
    for t in _bf_split3(-n2):
        Srows.append(t)
        Nrows.append(ones_n)
    for t in _bf_split3(-n1):
        Srows.append(ones_s)
        Nrows.append(t)
    X2 = np.stack([np.asarray(r, dtype=bf) for r in Srows])
    X1 = np.stack([np.asarray(r, dtype=bf) for r in Nrows])
    return X2, X1, n2.astype(np.float32), n1.astype(np.float32)


def _prep_inputs(xyz1, xyz2, points1, bf16mm=False):
    bfl = ml_dtypes.bfloat16
    xyz1 = np.asarray(xyz1, dtype=np.float32)
    xyz2 = np.asarray(xyz2, dtype=np.float32)
    points1 = np.asarray(points1, dtype=np.float32)
    in_maps = []
    for b in range(B):
        if bf16mm:
            X2, X1, n2, n1 = _host_matrices_bf16(xyz2[b], xyz1[b])
        else:
            X2, X1, n2, n1 = _host_matrices_fp8(xyz2[b], xyz1[b])
        # block table: row j holds points [16j, 16j+16): [2x, 2y, 2z, -n1]
        xb = np.empty((NBLK, 4, BPTS), dtype=np.float32)
        tx = (2.0 * xyz1[b]).astype(np.float32)       # [3, N]
        xb[:, 0:3, :] = tx.reshape(3, NBLK, BPTS).transpose(1, 0, 2)
        xb[:, 3, :] = -n1.reshape(NBLK, BPTS)
        # per-query [x, y, z, 1] and n2, laid out [p, chunk, ...]
        xq = np.empty((P, NCHUNK, 4), dtype=np.float32)
        q = xyz2[b].T.reshape(NCHUNK, P, 3)
        xq[:, :, 0:3] = q.transpose(1, 0, 2)
        xq[:, :, 3] = 1.0
        nq = np.ascontiguousarray(n2.reshape(NCHUNK, P).T)
        p1tb = np.ascontiguousarray(points1[b].T).astype(bfl)  # [N, D]
        in_maps.append({
            "x2m": X2, "x1m": X1,
            "xblk": xb.reshape(NBLK, BPTS * 4),
            "x2q": xq, "n2t": nq, "p1t": p1tb,
        })
    return in_maps


def _get_compiled(abl=()):
    key = tuple(sorted(abl))
    if key not in _COMPILED:
        _COMPILED[key] = _build_bass(abl=abl)
    return _COMPILED[key]


def kernel(xyz1, xyz2, points1):
    from concourse.bass_utils import run_bass_kernel_spmd

    nc = _get_compiled()
    in_maps = _prep_inputs(xyz1, xyz2, points1)
    res = run_bass_kernel_spmd(nc, in_maps, core_ids=list(range(B)))
    return np.stack(
        [r["outS"].astype(np.float32).T for r in res.results]
    )


if __name__ == "__main__":
    rng = np.random.default_rng(0)
    xyz1 = rng.standard_normal((B, 3, N), dtype=np.float32)
    xyz2 = rng.standard_normal((B, 3, S), dtype=np.float32)
    p1 = rng.standard_normal((B, D, N), dtype=np.float32)
    out = kernel(xyz1, xyz2, p1)
    print("out", out.shape, out.dtype)
    import test as T
    gt = T.np_reference_fp64(xyz1, xyz2, p1)
    diff = out.astype(np.float64) - gt.astype(np.float64)
    print("L2rel vs fp64:", np.linalg.norm(diff) / np.linalg.norm(gt))
